# revision 5
# baseline (speedup 1.0000x reference)
"""Trainium2 Bass kernel for nn_ExperimentalLayer9 (dense transformer layer).

Layer: x + gelu(attn(x) ) @ Wf with
  Q = split_heads(x), K = split_heads(x@Wk+bk), V = split_heads(x@Wv+bv)
  causal softmax (no 1/sqrt(d) scale), exact-erf gelu, residual add.

Sharding over 8 NeuronCores: 2 batch groups x 4-way head/tensor parallel.
Core c handles batch b=c//4 and heads [4r, 4r+4) with r=c%4.  Each core
computes K^T/V projections for its head slice, causal flash-style
attention in transposed-score layout, gelu, and a partial FF over its
1024-row slice of Wf.  A 4-rank ReduceScatter (bf16) sums the FF
partials within each batch group; each core adds the residual x rows for
its rank's 512-row shard and returns that shard.  The host reassembles
the [2, 2048, 1024] output.

All matmuls run in bf16 (fp32 PSUM accumulation); softmax/normalization
in fp32.  exp is computed without max-subtraction (scores are bounded:
std ~5, so exp stays well inside fp32/bf16 range) which avoids any
partition-axis max reduction.  The exp-sum l(q) is obtained for free by
appending a ones-column to V in the attention@V matmul; 1/l is then a
per-partition scalar multiply fused on the vector engine.
"""

import numpy as np
import ml_dtypes

import concourse.bass as bass
import concourse.mybir as mybir
import concourse.tile as tile
from concourse import bacc
from concourse import bass_utils

# Problem shapes (hardcoded per contest contract).
B, S, D, H, DHID = 2, 2048, 1024, 16, 4096
NCORES = 8
GROUP = 4              # cores per batch group
HPC = 4                # heads per core
DK = 64                # q/k head dim
DV = 256               # v head dim
DKS = HPC * DK         # 256  k-slice per core
DVS = HPC * DV         # 1024 v/hidden slice per core
ROWS = S // GROUP      # 512  output rows per core after ReduceScatter
NM = D // 128          # 8    contraction chunks over d_model
VSTRIDE = DV + 1       # 257  V columns per head incl. ones column

BF16 = mybir.dt.bfloat16
F32 = mybir.dt.float32
AF = mybir.ActivationFunctionType

bf16 = ml_dtypes.bfloat16

_compiled = None


def build_program():
    nc = bacc.Bacc(
        "TRN2",
        target_bir_lowering=False,
        debug=False,
        enable_asserts=True,
        num_devices=NCORES,
    )

    # Per-core inputs (values differ per core; program is SPMD-identical).
    xT = nc.dram_tensor("xT", [D, S], BF16, kind="ExternalInput").ap()
    qT = nc.dram_tensor("qT", [DKS, S], BF16, kind="ExternalInput").ap()
    xres = nc.dram_tensor("xres", [ROWS, D], F32, kind="ExternalInput").ap()
    wk = nc.dram_tensor("wk", [D, DKS], BF16, kind="ExternalInput").ap()
    wv = nc.dram_tensor("wv", [D, DVS], BF16, kind="ExternalInput").ap()
    wf = nc.dram_tensor("wf", [DVS, D], BF16, kind="ExternalInput").ap()
    bkb = nc.dram_tensor("bkb", [1, DKS], BF16, kind="ExternalInput").ap()
    bvb = nc.dram_tensor("bvb", [1, DVS], BF16, kind="ExternalInput").ap()
    maskt = nc.dram_tensor("maskt", [128, 128], BF16, kind="ExternalInput").ap()
    onesr = nc.dram_tensor("onesr", [1, 512], BF16, kind="ExternalInput").ap()
    out = nc.dram_tensor("out", [ROWS, D], F32, kind="ExternalOutput").ap()

    with tile.TileContext(nc) as tc:
        _body(nc, tc, xT, qT, xres, wk, wv, wf, bkb, bvb, maskt, onesr, out)

    nc.compile()
    return nc


def _body(nc, tc, xT, qT, xres, wk, wv, wf, bkb, bvb, maskt, onesr, out):
    NQT = S // 512     # 4  q tiles of 512
    NST = S // 128     # 16 s tiles of 128

    with (
        tc.tile_pool(name="const", bufs=1) as constp,
        tc.tile_pool(name="kv", bufs=1) as kvp,
        tc.tile_pool(name="onorm", bufs=1) as onp,
        tc.tile_pool(name="small", bufs=8) as smallp,
        tc.tile_pool(name="dram", bufs=1, space="DRAM") as dramp,
    ):
        # ---- constants ------------------------------------------------
        ones_sb = constp.tile([1, 512], BF16)
        nc.sync.dma_start(ones_sb[:], onesr[:])
        mask_sb = constp.tile([128, 128], BF16)
        nc.sync.dma_start(mask_sb[:], maskt[:])
        bk_sb = constp.tile([1, DKS], BF16)
        nc.sync.dma_start(bk_sb[:], bkb[:])
        bv_sb = constp.tile([1, DVS], BF16)
        nc.sync.dma_start(bv_sb[:], bvb[:])

        # [1024, n] DRAM -> [128, 8*n] SBUF, chunk m in cols [m*n,(m+1)*n)
        def load_chunked(pool, src, n):
            t = pool.tile([128, NM * n], src.dtype)
            nc.sync.dma_start(
                t[:].rearrange("p (m n) -> p m n", m=NM),
                src.rearrange("(m p) n -> p m n", p=128),
            )
            return t

        # live across proj+attention
        qT_sb = kvp.tile([128, 2 * S], BF16)
        nc.sync.dma_start(
            qT_sb[:].rearrange("p (m n) -> p m n", m=2),
            qT.rearrange("(m p) n -> p m n", p=128),
        )
        kt_sb = kvp.tile([128, 2 * S], BF16)   # K^T rows dk%128, chunk dk//128
        v_sb = kvp.tile([128, NST * HPC * VSTRIDE], BF16)
        onorm_sb = onp.tile([128, NST * DVS], BF16)  # o then gelu(o), [q-part, dv]

        # ---- projections ---------------------------------------------
        with (
            tc.tile_pool(name="projw", bufs=1) as pwp,
            tc.tile_pool(name="xt", bufs=1) as xtp,
            tc.tile_pool(name="psProj", bufs=4, space="PSUM") as psP,
        ):
            wk_sb = load_chunked(pwp, wk, DKS)
            wv_sb = load_chunked(pwp, wv, DVS)
            xT_sb = load_chunked(xtp, xT, S)

            # K^T[dk, s]: lhsT = Wk chunk [128m, 128dk], rhs = xT chunk [128m, 512s]
            for dkt in range(2):
                for st in range(NQT):
                    ps = psP.tile([128, 512], F32, tag="proj")
                    nc.tensor.matmul(
                        ps[:],
                        bk_sb[:, dkt * 128 : (dkt + 1) * 128],
                        ones_sb[:, 0:512],
                        start=True,
                        stop=False,
                    )
                    for m in range(NM):
                        nc.tensor.matmul(
                            ps[:],
                            wk_sb[:, m * DKS + dkt * 128 : m * DKS + dkt * 128 + 128],
                            xT_sb[:, m * S + st * 512 : m * S + st * 512 + 512],
                            start=False,
                            stop=(m == NM - 1),
                        )
                    nc.scalar.copy(
                        kt_sb[:, dkt * S + st * 512 : dkt * S + st * 512 + 512], ps[:]
                    )

            # V[s, dv] with a ones column per head (col 256 of each strip)
            nc.vector.memset(
                v_sb[:].rearrange("p (t h c) -> p t h c", t=NST, h=HPC)[:, :, :, DV],
                1.0,
            )
            for st in range(NST):
                for dvh in range(2):  # dv halves of 512 = heads (2*dvh, 2*dvh+1)
                    ps = psP.tile([128, 512], F32, tag="proj")
                    nc.tensor.matmul(
                        ps[:],
                        ones_sb[:, 0:128],
                        bv_sb[:, dvh * 512 : dvh * 512 + 512],
                        start=True,
                        stop=False,
                    )
                    for m in range(NM):
                        nc.tensor.matmul(
                            ps[:],
                            xT_sb[:, m * S + st * 128 : m * S + st * 128 + 128],
                            wv_sb[:, m * DVS + dvh * 512 : m * DVS + dvh * 512 + 512],
                            start=False,
                            stop=(m == NM - 1),
                        )
                    base = st * HPC * VSTRIDE
                    for hh in range(2):
                        h = 2 * dvh + hh
                        nc.scalar.copy(
                            v_sb[:, base + h * VSTRIDE : base + h * VSTRIDE + DV],
                            ps[:, hh * 256 : hh * 256 + 256],
                        )

        # ---- attention (per local head) ------------------------------
        # scores^T[k, q]: lhsT = K^T_h[d, k-tile], rhs = Q^T_h[d, q]
        with (
            tc.tile_pool(name="expp", bufs=2) as expp,
            tc.tile_pool(name="psSt", bufs=3, space="PSUM") as psS,
            tc.tile_pool(name="psAv", bufs=3, space="PSUM") as psV,
        ):
            for hl in range(HPC):
                po = 64 * (hl % 2)      # partition offset of this head's d rows
                co = (hl // 2) * S      # chunk col offset
                for j in range(NQT):
                    exps = expp.tile([128, 16 * 512], BF16, tag="expS")
                    nkt = 4 * j + 4
                    for kt in range(nkt):
                        t = kt - 4 * j   # >=0 on diagonal k-tiles
                        toff = max(t, 0) * 128
                        width = 512 - toff
                        q0 = j * 512 + toff
                        ps = psS.tile([128, 512], F32, tag="st")
                        nc.tensor.matmul(
                            ps[:, 0:width],
                            kt_sb[po : po + 64, co + kt * 128 : co + kt * 128 + 128],
                            qT_sb[po : po + 64, co + q0 : co + q0 + width],
                            start=True,
                            stop=True,
                        )
                        nc.scalar.activation(
                            exps[:, kt * 512 + toff : kt * 512 + toff + width],
                            ps[:, 0:width],
                            AF.Exp,
                        )
                        if t >= 0:  # mask the diagonal 128x128 block
                            blk = exps[:, kt * 512 + toff : kt * 512 + toff + 128]
                            nc.vector.tensor_mul(blk, blk, mask_sb[:])
                    # attn @ [V | 1] per 128-wide q subtile
                    for sq in range(4):
                        i = 4 * j + sq
                        pso = psV.tile([128, VSTRIDE], F32, tag="av")
                        for kt in range(i + 1):
                            vb = kt * HPC * VSTRIDE + hl * VSTRIDE
                            nc.tensor.matmul(
                                pso[:],
                                exps[:, kt * 512 + sq * 128 : kt * 512 + sq * 128 + 128],
                                v_sb[:, vb : vb + VSTRIDE],
                                start=(kt == 0),
                                stop=(kt == i),
                            )
                        recip = smallp.tile([128, 1], F32, tag="recip")
                        nc.vector.reciprocal(recip[:], pso[:, DV : DV + 1])
                        nc.vector.tensor_scalar_mul(
                            onorm_sb[:, i * DVS + hl * DV : i * DVS + (hl + 1) * DV],
                            pso[:, 0:DV],
                            recip[:],
                        )

        # ---- gelu (exact erf) in place -------------------------------
        for c in range(NST):
            nc.scalar.activation(
                onorm_sb[:, c * DVS : (c + 1) * DVS],
                onorm_sb[:, c * DVS : (c + 1) * DVS],
                AF.Gelu,
            )

        # ---- transpose + FF partial ----------------------------------
        partial_d = dramp.tile([S, D], BF16)
        with (
            tc.tile_pool(name="ffw", bufs=1) as ffwp,
            tc.tile_pool(name="got", bufs=1) as gotp,
            tc.tile_pool(name="ffout", bufs=3) as ffoutp,
            tc.tile_pool(name="psFf", bufs=2, space="PSUM") as psF,
        ):
            wf_sb = load_chunked(ffwp, wf, D)
            # gelu(o) -> [h, q] layout via DMA xbar transpose
            got_sb = gotp.tile([128, NM * S], BF16)  # chunk hc at [hc*S,(hc+1)*S)
            for c in range(NST):
                for hc in range(NM):
                    nc.sync.dma_start_transpose(
                        got_sb[:, hc * S + c * 128 : hc * S + c * 128 + 128],
                        onorm_sb[:, c * DVS + hc * 128 : c * DVS + hc * 128 + 128],
                    )
            # partial_ff[q, n] = gelu(o)^T.T @ Wf_slice
            for c in range(NST):
                ps0 = psF.tile([128, 512], F32, tag="ff0")
                ps1 = psF.tile([128, 512], F32, tag="ff1")
                for hc in range(NM):
                    lhsT = got_sb[:, hc * S + c * 128 : hc * S + c * 128 + 128]
                    nc.tensor.matmul(
                        ps0[:], lhsT, wf_sb[:, hc * D : hc * D + 512],
                        start=(hc == 0), stop=(hc == NM - 1),
                    )
                    nc.tensor.matmul(
                        ps1[:], lhsT, wf_sb[:, hc * D + 512 : hc * D + 1024],
                        start=(hc == 0), stop=(hc == NM - 1),
                    )
                fo = ffoutp.tile([128, D], BF16, tag="ffout")
                nc.vector.tensor_copy(fo[:, 0:512], ps0[:])
                nc.vector.tensor_copy(fo[:, 512:1024], ps1[:])
                nc.sync.dma_start(partial_d[c * 128 : (c + 1) * 128, :], fo[:])

        # ---- ReduceScatter within batch group + residual -------------
        rs_d = dramp.tile([ROWS, D], BF16)
        nc.gpsimd.collective_compute(
            "ReduceScatter",
            mybir.AluOpType.add,
            replica_groups=[[0, 1, 2, 3], [4, 5, 6, 7]],
            ins=[partial_d.opt()],
            outs=[rs_d.opt()],
        )
        with tc.tile_pool(name="res", bufs=2) as resp:
            for c in range(ROWS // 128):
                xr = resp.tile([128, D], F32, tag="xr")
                nc.sync.dma_start(xr[:], xres[c * 128 : (c + 1) * 128, :])
                rb = resp.tile([128, D], BF16, tag="rb")
                nc.sync.dma_start(rb[:], rs_d[c * 128 : (c + 1) * 128, :])
                rf = resp.tile([128, D], F32, tag="rf")
                nc.vector.tensor_copy(rf[:], rb[:])
                nc.vector.tensor_add(xr[:], xr[:], rf[:])
                nc.sync.dma_start(out[c * 128 : (c + 1) * 128, :], xr[:])


def make_in_maps(x, Wk, bk, Wv, bv, Wf, bf):
    """Host-side sharding: returns the per-core input dict list."""
    x = np.asarray(x, np.float32)
    mask = np.tril(np.ones((128, 128), np.float32)).T  # mask[k,q]=1 iff k<=q
    in_maps = []
    for c in range(NCORES):
        b, r = c // GROUP, c % GROUP
        xb = x[b]                                    # [S, D]
        xT = np.ascontiguousarray(xb.T).astype(bf16)
        qTs = xT[DKS * r : DKS * (r + 1)]            # heads 4r..4r+3 rows
        in_maps.append({
            "xT": xT,
            "qT": np.ascontiguousarray(qTs),
            "xres": np.ascontiguousarray(
                xb[ROWS * r : ROWS * (r + 1)] + bf[None, :].astype(np.float32)
            ),
            "wk": np.ascontiguousarray(Wk[:, DKS * r : DKS * (r + 1)]).astype(bf16),
            "wv": np.ascontiguousarray(Wv[:, DVS * r : DVS * (r + 1)]).astype(bf16),
            "wf": np.ascontiguousarray(Wf[DVS * r : DVS * (r + 1), :]).astype(bf16),
            "bkb": bk[None, DKS * r : DKS * (r + 1)].astype(bf16),
            "bvb": bv[None, DVS * r : DVS * (r + 1)].astype(bf16),
            "maskt": mask.astype(bf16),
            "onesr": np.ones((1, 512), bf16),
        })
    return in_maps


def assemble(results):
    """[8 x [512,1024]] core outputs -> [2,2048,1024]."""
    out = np.empty((B, S, D), np.float32)
    for c in range(NCORES):
        b, r = c // GROUP, c % GROUP
        out[b, ROWS * r : ROWS * (r + 1), :] = results[c]["out"]
    return out


def kernel(x, Wk, bk, Wv, bv, Wf, bf, _trace=False, _trace_cores=None):
    global _compiled
    if _compiled is None:
        _compiled = build_program()
    nc = _compiled
    in_maps = make_in_maps(x, Wk, bk, Wv, bv, Wf, bf)
    res = bass_utils.run_bass_kernel_spmd(
        nc,
        in_maps,
        core_ids=list(range(NCORES)),
        trace=_trace,
        trace_cores=_trace_cores,
    )
    out = assemble(res.results)
    kernel.last_result = res
    return out


# revision 10
# speedup vs baseline: 1.2558x; 1.2558x over previous
"""Trainium2 Bass kernel for nn_ExperimentalLayer9 (dense transformer layer).

Layer: x + gelu(attn(x) ) @ Wf with
  Q = split_heads(x), K = split_heads(x@Wk+bk), V = split_heads(x@Wv+bv)
  causal softmax (no 1/sqrt(d) scale), exact-erf gelu, residual add.

Sharding over 8 NeuronCores: 2 batch groups x 4-way head/tensor parallel.
Core c handles batch b=c//4 and heads [4r, 4r+4) with r=c%4.  Each core
computes K^T/V projections for its head slice, causal flash-style
attention in transposed-score layout, gelu, and a partial FF over its
1024-row slice of Wf.  A 4-rank ReduceScatter (bf16) sums the FF
partials within each batch group; each core adds the residual x rows for
its rank's 512-row shard and returns that shard.  The host reassembles
the [2, 2048, 1024] output.

All matmuls run in bf16 (fp32 PSUM accumulation); softmax/normalization
in fp32.  exp is computed without max-subtraction (scores are bounded:
std ~5, so exp stays well inside fp32/bf16 range) which avoids any
partition-axis max reduction.  The exp-sum l(q) is obtained for free by
appending a ones-column to V in the attention@V matmul; 1/l is then a
per-partition scalar multiply fused on the vector engine.
"""

import numpy as np
import ml_dtypes

import concourse.bass as bass
import concourse.mybir as mybir
import concourse.tile as tile
from concourse import bacc
from concourse import bass_utils

# Problem shapes (hardcoded per contest contract).
B, S, D, H, DHID = 2, 2048, 1024, 16, 4096
NCORES = 8
GROUP = 4              # cores per batch group
HPC = 4                # heads per core
DK = 64                # q/k head dim
DV = 256               # v head dim
DKS = HPC * DK         # 256  k-slice per core
DVS = HPC * DV         # 1024 v/hidden slice per core
ROWS = S // GROUP      # 512  output rows per core after ReduceScatter
NM = D // 128          # 8    contraction chunks over d_model
VSTRIDE = DV + 1       # 257  V columns per head incl. ones column

BF16 = mybir.dt.bfloat16
F32 = mybir.dt.float32
AF = mybir.ActivationFunctionType

bf16 = ml_dtypes.bfloat16

_compiled = None


def build_program():
    nc = bacc.Bacc(
        "TRN2",
        target_bir_lowering=False,
        debug=False,
        enable_asserts=True,
        num_devices=NCORES,
    )

    # Per-core inputs (values differ per core; program is SPMD-identical).
    xT = nc.dram_tensor("xT", [D, S], BF16, kind="ExternalInput").ap()
    qT = nc.dram_tensor("qT", [DKS, S], BF16, kind="ExternalInput").ap()
    xres = nc.dram_tensor("xres", [ROWS, D], F32, kind="ExternalInput").ap()
    wk = nc.dram_tensor("wk", [D, DKS], BF16, kind="ExternalInput").ap()
    wv = nc.dram_tensor("wv", [D, DVS], BF16, kind="ExternalInput").ap()
    wf = nc.dram_tensor("wf", [DVS, D], BF16, kind="ExternalInput").ap()
    bkb = nc.dram_tensor("bkb", [1, DKS], BF16, kind="ExternalInput").ap()
    bvb = nc.dram_tensor("bvb", [1, DVS], BF16, kind="ExternalInput").ap()
    maskt = nc.dram_tensor("maskt", [128, 128], BF16, kind="ExternalInput").ap()
    ident = nc.dram_tensor("ident", [128, 128], BF16, kind="ExternalInput").ap()
    onesr = nc.dram_tensor("onesr", [1, 512], BF16, kind="ExternalInput").ap()
    out = nc.dram_tensor("out", [ROWS, D], F32, kind="ExternalOutput").ap()

    with tile.TileContext(nc) as tc:
        _body(nc, tc, xT, qT, xres, wk, wv, wf, bkb, bvb, maskt, ident, onesr, out)

    nc.compile()
    return nc


def _body(nc, tc, xT, qT, xres, wk, wv, wf, bkb, bvb, maskt, ident, onesr, out):
    NST = S // 128     # 16 s tiles of 128
    NQT2 = S // 1024   # 2  q tiles of 1024

    with (
        tc.tile_pool(name="const", bufs=1) as constp,
        tc.tile_pool(name="kv", bufs=1) as kvp,
        tc.tile_pool(name="got", bufs=1) as gotp,
        tc.tile_pool(name="small", bufs=8) as smallp,
        tc.tile_pool(name="dram", bufs=1, space="DRAM") as dramp,
    ):
        # ---- constants ------------------------------------------------
        ones_sb = constp.tile([1, 512], BF16)
        nc.sync.dma_start(ones_sb[:], onesr[:])
        mask_sb = constp.tile([128, 128], BF16)
        nc.sync.dma_start(mask_sb[:], maskt[:])
        ident_sb = constp.tile([128, 128], BF16)
        nc.sync.dma_start(ident_sb[:], ident[:])
        bk_sb = constp.tile([1, DKS], BF16)
        nc.sync.dma_start(bk_sb[:], bkb[:])
        bv_sb = constp.tile([1, DVS], BF16)
        nc.sync.dma_start(bv_sb[:], bvb[:])

        # [1024, n] DRAM -> [128, 8*n] SBUF, chunk m in cols [m*n,(m+1)*n)
        def load_chunked(pool, src, n):
            t = pool.tile([128, NM * n], src.dtype)
            nc.sync.dma_start(
                t[:].rearrange("p (m n) -> p m n", m=NM),
                src.rearrange("(m p) n -> p m n", p=128),
            )
            return t

        # live across proj+attention
        qT_sb = kvp.tile([128, 2 * S], BF16)
        nc.sync.dma_start(
            qT_sb[:].rearrange("p (m n) -> p m n", m=2),
            qT.rearrange("(m p) n -> p m n", p=128),
        )
        kt_sb = kvp.tile([128, 2 * S], BF16)   # K^T rows dk%128, chunk dk//128
        v_sb = kvp.tile([128, NST * HPC * VSTRIDE], BF16)
        # gelu(o)^T staging: [dv_local(hc) part, hc-major x q] bf16
        got_sb = gotp.tile([128, NM * S], BF16)

        # ---- projections ---------------------------------------------
        with (
            tc.tile_pool(name="projw", bufs=1) as pwp,
            tc.tile_pool(name="xt", bufs=1) as xtp,
            tc.tile_pool(name="psProj", bufs=4, space="PSUM") as psP,
        ):
            wk_sb = load_chunked(pwp, wk, DKS)
            wv_sb = load_chunked(pwp, wv, DVS)
            xT_sb = load_chunked(xtp, xT, S)

            # K^T[dk, s]: lhsT = Wk chunk [128m, 128dk], rhs = xT chunk [128m, 512s]
            for dkt in range(2):
                for st in range(4):
                    ps = psP.tile([128, 512], F32, tag="proj")
                    nc.tensor.matmul(
                        ps[:],
                        bk_sb[:, dkt * 128 : (dkt + 1) * 128],
                        ones_sb[:, 0:512],
                        start=True,
                        stop=False,
                    )
                    for m in range(NM):
                        nc.tensor.matmul(
                            ps[:],
                            wk_sb[:, m * DKS + dkt * 128 : m * DKS + dkt * 128 + 128],
                            xT_sb[:, m * S + st * 512 : m * S + st * 512 + 512],
                            start=False,
                            stop=(m == NM - 1),
                        )
                    nc.scalar.copy(
                        kt_sb[:, dkt * S + st * 512 : dkt * S + st * 512 + 512], ps[:]
                    )

            # V[s, dv] with a ones column per head (col 256 of each strip)
            nc.vector.memset(
                v_sb[:].rearrange("p (t h c) -> p t h c", t=NST, h=HPC)[:, :, :, DV],
                1.0,
            )
            for st in range(NST):
                for dvh in range(2):  # dv halves of 512 = heads (2*dvh, 2*dvh+1)
                    ps = psP.tile([128, 512], F32, tag="proj")
                    nc.tensor.matmul(
                        ps[:],
                        ones_sb[:, 0:128],
                        bv_sb[:, dvh * 512 : dvh * 512 + 512],
                        start=True,
                        stop=False,
                    )
                    for m in range(NM):
                        nc.tensor.matmul(
                            ps[:],
                            xT_sb[:, m * S + st * 128 : m * S + st * 128 + 128],
                            wv_sb[:, m * DVS + dvh * 512 : m * DVS + dvh * 512 + 512],
                            start=False,
                            stop=(m == NM - 1),
                        )
                    base = st * HPC * VSTRIDE
                    for hh in range(2):
                        h = 2 * dvh + hh
                        nc.scalar.copy(
                            v_sb[:, base + h * VSTRIDE : base + h * VSTRIDE + DV],
                            ps[:, hh * 256 : hh * 256 + 256],
                        )

        # ---- attention (per local head) ------------------------------
        # scores^T[k, q] in 1024-wide q tiles; exp without max-subtraction;
        # o = attn @ [V|1] per 128-q subtile, fused 1/l on DVE, then PE
        # transpose into got_sb (pre-gelu).
        with (
            tc.tile_pool(name="expp", bufs=2) as expp,
            tc.tile_pool(name="otile", bufs=3) as otp,
            tc.tile_pool(name="psSt", bufs=2, space="PSUM") as psS,
            tc.tile_pool(name="psAv", bufs=2, space="PSUM") as psV,
            tc.tile_pool(name="psTp", bufs=2, space="PSUM") as psT,
        ):
            for hl in range(HPC):
                po = 64 * (hl % 2)      # partition offset of this head's d rows
                co = (hl // 2) * S      # chunk col offset
                for j in range(NQT2):   # 1024-wide q tiles
                    exps = expp.tile([128, 16 * 1024], BF16, tag="expS")
                    nkt = 8 * j + 8
                    for kt in range(nkt):
                        t = kt - 8 * j   # >=0 on diagonal k-tiles
                        toff = max(t, 0) * 128
                        width = 1024 - toff
                        q0 = j * 1024 + toff
                        ps = psS.tile([128, 1024], F32, tag="st")
                        # matmuls may not cross a PSUM bank: split at col 512
                        lo_w = max(0, 512 - toff)
                        if lo_w:
                            nc.tensor.matmul(
                                ps[:, toff : toff + lo_w],
                                kt_sb[po : po + 64, co + kt * 128 : co + kt * 128 + 128],
                                qT_sb[po : po + 64, co + q0 : co + q0 + lo_w],
                                start=True,
                                stop=True,
                            )
                        nc.tensor.matmul(
                            ps[:, max(toff, 512) : 1024],
                            kt_sb[po : po + 64, co + kt * 128 : co + kt * 128 + 128],
                            qT_sb[po : po + 64, co + j * 1024 + max(toff, 512) : co + (j + 1) * 1024],
                            start=True,
                            stop=True,
                        )
                        nc.scalar.activation(
                            exps[:, kt * 1024 + toff : (kt + 1) * 1024],
                            ps[:, toff:1024],
                            AF.Exp,
                        )
                        if t >= 0:  # mask the diagonal 128x128 block
                            blk = exps[:, kt * 1024 + toff : kt * 1024 + toff + 128]
                            nc.vector.tensor_mul(blk, blk, mask_sb[:])
                    # attn @ [V | 1] per 128-wide q subtile
                    for sq in range(8):
                        i = 8 * j + sq
                        pso = psV.tile([128, VSTRIDE], F32, tag="av")
                        for kt in range(i + 1):
                            vb = kt * HPC * VSTRIDE + hl * VSTRIDE
                            nc.tensor.matmul(
                                pso[:],
                                exps[:, kt * 1024 + sq * 128 : kt * 1024 + sq * 128 + 128],
                                v_sb[:, vb : vb + VSTRIDE],
                                start=(kt == 0),
                                stop=(kt == i),
                            )
                        recip = smallp.tile([128, 1], F32, tag="recip")
                        nc.vector.reciprocal(recip[:], pso[:, DV : DV + 1])
                        ot = otp.tile([128, DV], BF16, tag="ot")
                        nc.vector.tensor_scalar_mul(ot[:], pso[:, 0:DV], recip[:])
                        # transpose o tile into got_sb (pre-gelu)
                        for half in range(2):
                            hc = 2 * hl + half
                            pt = psT.tile([128, 128], BF16, tag="tp")
                            nc.tensor.transpose(
                                pt[:], ot[:, half * 128 : half * 128 + 128], ident_sb[:]
                            )
                            nc.vector.tensor_copy(
                                got_sb[:, hc * S + i * 128 : hc * S + i * 128 + 128],
                                pt[:],
                            )

        # ---- gelu (exact erf) in place on transposed layout ----------
        for hc in range(NM):
            nc.scalar.activation(
                got_sb[:, hc * S : (hc + 1) * S],
                got_sb[:, hc * S : (hc + 1) * S],
                AF.Gelu,
            )

        # ---- FF partial + chunked ReduceScatter + residual -----------
        with (
            tc.tile_pool(name="ffw", bufs=1) as ffwp,
            tc.tile_pool(name="ffout", bufs=3) as ffoutp,
            tc.tile_pool(name="res", bufs=2) as resp,
            tc.tile_pool(name="psFf", bufs=2, space="PSUM") as psF,
        ):
            wf_sb = load_chunked(ffwp, wf, D)
            for g in range(4):
                partial_d = dramp.tile([512, D], BF16, tag=f"part{g}")
                for cc in range(4):
                    c = 4 * g + cc
                    ps0 = psF.tile([128, 512], F32, tag="ff0")
                    ps1 = psF.tile([128, 512], F32, tag="ff1")
                    for hc in range(NM):
                        lhsT = got_sb[:, hc * S + c * 128 : hc * S + c * 128 + 128]
                        nc.tensor.matmul(
                            ps0[:], lhsT, wf_sb[:, hc * D : hc * D + 512],
                            start=(hc == 0), stop=(hc == NM - 1),
                        )
                        nc.tensor.matmul(
                            ps1[:], lhsT, wf_sb[:, hc * D + 512 : hc * D + 1024],
                            start=(hc == 0), stop=(hc == NM - 1),
                        )
                    fo = ffoutp.tile([128, D], BF16, tag="ffout")
                    nc.vector.tensor_copy(fo[:, 0:512], ps0[:])
                    nc.vector.tensor_copy(fo[:, 512:1024], ps1[:])
                    nc.sync.dma_start(partial_d[cc * 128 : (cc + 1) * 128, :], fo[:])
                rs_d = dramp.tile([128, D], BF16, tag=f"rs{g}")
                nc.gpsimd.collective_compute(
                    "ReduceScatter",
                    mybir.AluOpType.add,
                    replica_groups=[[0, 1, 2, 3], [4, 5, 6, 7]],
                    ins=[partial_d.opt()],
                    outs=[rs_d.opt()],
                )
                xr = resp.tile([128, D], F32, tag="xr")
                nc.sync.dma_start(xr[:], xres[g * 128 : (g + 1) * 128, :])
                rb = resp.tile([128, D], BF16, tag="rb")
                nc.sync.dma_start(rb[:], rs_d[:])
                rf = resp.tile([128, D], F32, tag="rf")
                nc.vector.tensor_copy(rf[:], rb[:])
                nc.vector.tensor_add(xr[:], xr[:], rf[:])
                nc.sync.dma_start(out[g * 128 : (g + 1) * 128, :], xr[:])


def make_in_maps(x, Wk, bk, Wv, bv, Wf, bf):
    """Host-side sharding: returns the per-core input dict list."""
    x = np.asarray(x, np.float32)
    mask = np.tril(np.ones((128, 128), np.float32)).T  # mask[k,q]=1 iff k<=q
    in_maps = []
    for c in range(NCORES):
        b, r = c // GROUP, c % GROUP
        xb = x[b]                                    # [S, D]
        xT = np.ascontiguousarray(xb.T).astype(bf16)
        qTs = xT[DKS * r : DKS * (r + 1)]            # heads 4r..4r+3 rows
        # chunked RS: core (b,r) ends up with rows {512g+128r+[0,128)}
        xres = np.concatenate(
            [xb[512 * g + 128 * r : 512 * g + 128 * r + 128] for g in range(4)]
        ) + bf[None, :].astype(np.float32)
        in_maps.append({
            "xT": xT,
            "qT": np.ascontiguousarray(qTs),
            "xres": np.ascontiguousarray(xres),
            "wk": np.ascontiguousarray(Wk[:, DKS * r : DKS * (r + 1)]).astype(bf16),
            "wv": np.ascontiguousarray(Wv[:, DVS * r : DVS * (r + 1)]).astype(bf16),
            "wf": np.ascontiguousarray(Wf[DVS * r : DVS * (r + 1), :]).astype(bf16),
            "bkb": bk[None, DKS * r : DKS * (r + 1)].astype(bf16),
            "bvb": bv[None, DVS * r : DVS * (r + 1)].astype(bf16),
            "maskt": mask.astype(bf16),
            "ident": np.eye(128, dtype=np.float32).astype(bf16),
            "onesr": np.ones((1, 512), bf16),
        })
    return in_maps


def assemble(results):
    """[8 x [512,1024]] core outputs -> [2,2048,1024]."""
    out = np.empty((B, S, D), np.float32)
    for c in range(NCORES):
        b, r = c // GROUP, c % GROUP
        for g in range(4):
            out[b, 512 * g + 128 * r : 512 * g + 128 * r + 128, :] = results[c][
                "out"
            ][128 * g : 128 * (g + 1)]
    return out


def kernel(x, Wk, bk, Wv, bv, Wf, bf, _trace=False, _trace_cores=None):
    global _compiled
    if _compiled is None:
        _compiled = build_program()
    nc = _compiled
    in_maps = make_in_maps(x, Wk, bk, Wv, bv, Wf, bf)
    res = bass_utils.run_bass_kernel_spmd(
        nc,
        in_maps,
        core_ids=list(range(NCORES)),
        trace=_trace,
        trace_cores=_trace_cores,
    )
    out = assemble(res.results)
    kernel.last_result = res
    return out


# revision 11
# speedup vs baseline: 1.2821x; 1.0210x over previous
"""Trainium2 Bass kernel for nn_ExperimentalLayer9 (dense transformer layer).

Layer: x + gelu(attn(x) ) @ Wf with
  Q = split_heads(x), K = split_heads(x@Wk+bk), V = split_heads(x@Wv+bv)
  causal softmax (no 1/sqrt(d) scale), exact-erf gelu, residual add.

Sharding over 8 NeuronCores: 2 batch groups x 4-way head/tensor parallel.
Core c handles batch b=c//4 and heads [4r, 4r+4) with r=c%4.  Each core
computes K^T/V projections for its head slice, causal flash-style
attention in transposed-score layout, gelu, and a partial FF over its
1024-row slice of Wf.  A 4-rank ReduceScatter (bf16) sums the FF
partials within each batch group; each core adds the residual x rows for
its rank's 512-row shard and returns that shard.  The host reassembles
the [2, 2048, 1024] output.

All matmuls run in bf16 (fp32 PSUM accumulation); softmax/normalization
in fp32.  exp is computed without max-subtraction (scores are bounded:
std ~5, so exp stays well inside fp32/bf16 range) which avoids any
partition-axis max reduction.  The exp-sum l(q) is obtained for free by
appending a ones-column to V in the attention@V matmul; 1/l is then a
per-partition scalar multiply fused on the vector engine.
"""

import numpy as np
import ml_dtypes

import concourse.bass as bass
import concourse.mybir as mybir
import concourse.tile as tile
from concourse import bacc
from concourse import bass_utils

# Problem shapes (hardcoded per contest contract).
B, S, D, H, DHID = 2, 2048, 1024, 16, 4096
NCORES = 8
GROUP = 4              # cores per batch group
HPC = 4                # heads per core
DK = 64                # q/k head dim
DV = 256               # v head dim
DKS = HPC * DK         # 256  k-slice per core
DVS = HPC * DV         # 1024 v/hidden slice per core
ROWS = S // GROUP      # 512  output rows per core after ReduceScatter
NM = D // 128          # 8    contraction chunks over d_model
VSTRIDE = DV + 1       # 257  V columns per head incl. ones column

BF16 = mybir.dt.bfloat16
F32 = mybir.dt.float32
AF = mybir.ActivationFunctionType

bf16 = ml_dtypes.bfloat16

_compiled = None


def build_program():
    nc = bacc.Bacc(
        "TRN2",
        target_bir_lowering=False,
        debug=False,
        enable_asserts=True,
        num_devices=NCORES,
    )

    # Per-core inputs (values differ per core; program is SPMD-identical).
    xT = nc.dram_tensor("xT", [D, S], BF16, kind="ExternalInput").ap()
    qT = nc.dram_tensor("qT", [DKS, S], BF16, kind="ExternalInput").ap()
    xres = nc.dram_tensor("xres", [ROWS, D], F32, kind="ExternalInput").ap()
    wk = nc.dram_tensor("wk", [D, DKS], BF16, kind="ExternalInput").ap()
    wv = nc.dram_tensor("wv", [D, DVS], BF16, kind="ExternalInput").ap()
    wf = nc.dram_tensor("wf", [DVS, D], BF16, kind="ExternalInput").ap()
    bkb = nc.dram_tensor("bkb", [1, DKS], BF16, kind="ExternalInput").ap()
    bvb = nc.dram_tensor("bvb", [1, DVS], BF16, kind="ExternalInput").ap()
    maskt = nc.dram_tensor("maskt", [128, 128], BF16, kind="ExternalInput").ap()
    ident = nc.dram_tensor("ident", [128, 128], BF16, kind="ExternalInput").ap()
    onesr = nc.dram_tensor("onesr", [1, 512], BF16, kind="ExternalInput").ap()
    out = nc.dram_tensor("out", [ROWS, D], F32, kind="ExternalOutput").ap()

    with tile.TileContext(nc) as tc:
        _body(nc, tc, xT, qT, xres, wk, wv, wf, bkb, bvb, maskt, ident, onesr, out)

    nc.compile()
    return nc


def _body(nc, tc, xT, qT, xres, wk, wv, wf, bkb, bvb, maskt, ident, onesr, out):
    NST = S // 128     # 16 s tiles of 128
    NQT2 = S // 1024   # 2  q tiles of 1024

    # Engine split: nc.scalar issues every plain DMA (HWDGE via ACT) so the
    # Sync queue carries only the xbar transposes -- no head-of-line blocking
    # between the store stream and collective-gated loads.
    dma = nc.scalar

    with (
        tc.tile_pool(name="const", bufs=1) as constp,
        tc.tile_pool(name="kv", bufs=1) as kvp,
        tc.tile_pool(name="got", bufs=1) as gotp,
        tc.tile_pool(name="small", bufs=8) as smallp,
        tc.tile_pool(name="dram", bufs=1, space="DRAM") as dramp,
    ):
        # ---- constants ------------------------------------------------
        ones_sb = constp.tile([1, 512], BF16)
        dma.dma_start(ones_sb[:], onesr[:])
        mask_sb = constp.tile([128, 128], BF16)
        dma.dma_start(mask_sb[:], maskt[:])
        bk_sb = constp.tile([1, DKS], BF16)
        dma.dma_start(bk_sb[:], bkb[:])
        bv_sb = constp.tile([1, DVS], BF16)
        dma.dma_start(bv_sb[:], bvb[:])

        # [1024, n] DRAM -> [128, 8*n] SBUF, chunk m in cols [m*n,(m+1)*n),
        # one DMA per chunk so consumers start as soon as chunk 0 lands
        def load_chunked(pool, src, n):
            t = pool.tile([128, NM * n], src.dtype)
            for m in range(NM):
                dma.dma_start(
                    t[:, m * n : (m + 1) * n],
                    src[m * 128 : (m + 1) * 128, :],
                )
            return t

        # live across proj+attention
        qT_sb = kvp.tile([128, 2 * S], BF16)
        for m in range(2):
            dma.dma_start(
                qT_sb[:, m * S : (m + 1) * S], qT[m * 128 : (m + 1) * 128, :]
            )
        kt_sb = kvp.tile([128, 2 * S], BF16)   # K^T rows dk%128, chunk dk//128
        v_sb = kvp.tile([128, NST * HPC * VSTRIDE], BF16)
        # gelu(o)^T staging: [dv_local(hc) part, hc-major x q] bf16
        got_sb = gotp.tile([128, NM * S], BF16)

        # ---- projections ---------------------------------------------
        with (
            tc.tile_pool(name="projw", bufs=1) as pwp,
            tc.tile_pool(name="xt", bufs=1) as xtp,
            tc.tile_pool(name="psProj", bufs=4, space="PSUM") as psP,
        ):
            wk_sb = load_chunked(pwp, wk, DKS)
            xT_sb = load_chunked(xtp, xT, S)
            wv_sb = load_chunked(pwp, wv, DVS)

            # K^T[dk, s]: lhsT = Wk chunk [128m, 128dk], rhs = xT chunk [128m, 512s]
            for dkt in range(2):
                for st in range(4):
                    ps = psP.tile([128, 512], F32, tag="proj")
                    nc.tensor.matmul(
                        ps[:],
                        bk_sb[:, dkt * 128 : (dkt + 1) * 128],
                        ones_sb[:, 0:512],
                        start=True,
                        stop=False,
                    )
                    for m in range(NM):
                        nc.tensor.matmul(
                            ps[:],
                            wk_sb[:, m * DKS + dkt * 128 : m * DKS + dkt * 128 + 128],
                            xT_sb[:, m * S + st * 512 : m * S + st * 512 + 512],
                            start=False,
                            stop=(m == NM - 1),
                        )
                    nc.scalar.copy(
                        kt_sb[:, dkt * S + st * 512 : dkt * S + st * 512 + 512], ps[:]
                    )

            # V[s, dv] with a ones column per head (col 256 of each strip)
            nc.vector.memset(
                v_sb[:].rearrange("p (t h c) -> p t h c", t=NST, h=HPC)[:, :, :, DV],
                1.0,
            )
            for st in range(NST):
                for dvh in range(2):  # dv halves of 512 = heads (2*dvh, 2*dvh+1)
                    ps = psP.tile([128, 512], F32, tag="proj")
                    nc.tensor.matmul(
                        ps[:],
                        ones_sb[:, 0:128],
                        bv_sb[:, dvh * 512 : dvh * 512 + 512],
                        start=True,
                        stop=False,
                    )
                    for m in range(NM):
                        nc.tensor.matmul(
                            ps[:],
                            xT_sb[:, m * S + st * 128 : m * S + st * 128 + 128],
                            wv_sb[:, m * DVS + dvh * 512 : m * DVS + dvh * 512 + 512],
                            start=False,
                            stop=(m == NM - 1),
                        )
                    base = st * HPC * VSTRIDE
                    for hh in range(2):
                        h = 2 * dvh + hh
                        nc.scalar.copy(
                            v_sb[:, base + h * VSTRIDE : base + h * VSTRIDE + DV],
                            ps[:, hh * 256 : hh * 256 + 256],
                        )

        # ---- attention (per local head) ------------------------------
        # scores^T[k, q] in 1024-wide q tiles; exp without max-subtraction.
        # PE order interleaves AV groups between diagonal score tiles so the
        # tensor engine never drains while ACT computes exp.
        # o tiles are transposed into got_sb by xbar DMA on the Sync queue.
        with (
            tc.tile_pool(name="expp", bufs=2) as expp,
            tc.tile_pool(name="otile", bufs=4) as otp,
            tc.tile_pool(name="psSt", bufs=3, space="PSUM") as psS,
            tc.tile_pool(name="psAv", bufs=2, space="PSUM") as psV,
        ):
            for hl in range(HPC):
                po = 64 * (hl % 2)      # partition offset of this head's d rows
                co = (hl // 2) * S      # chunk col offset

                def st_tile(j, kt):
                    t = kt - 8 * j   # >=0 on diagonal k-tiles
                    toff = max(t, 0) * 128
                    q0 = j * 1024 + toff
                    ps = psS.tile([128, 1024], F32, tag="st")
                    lo_w = max(0, 512 - toff)
                    if lo_w:
                        nc.tensor.matmul(
                            ps[:, toff : toff + lo_w],
                            kt_sb[po : po + 64, co + kt * 128 : co + kt * 128 + 128],
                            qT_sb[po : po + 64, co + q0 : co + q0 + lo_w],
                            start=True,
                            stop=True,
                        )
                    nc.tensor.matmul(
                        ps[:, max(toff, 512) : 1024],
                        kt_sb[po : po + 64, co + kt * 128 : co + kt * 128 + 128],
                        qT_sb[po : po + 64, co + j * 1024 + max(toff, 512) : co + (j + 1) * 1024],
                        start=True,
                        stop=True,
                    )
                    nc.scalar.activation(
                        exps[:, kt * 1024 + toff : (kt + 1) * 1024],
                        ps[:, toff:1024],
                        AF.Exp,
                    )
                    if t >= 0:  # mask the diagonal 128x128 block
                        blk = exps[:, kt * 1024 + toff : kt * 1024 + toff + 128]
                        nc.vector.tensor_mul(blk, blk, mask_sb[:])

                def av_tile(j, sq):
                    i = 8 * j + sq
                    pso = psV.tile([128, VSTRIDE], F32, tag="av")
                    for kt in range(i + 1):
                        vb = kt * HPC * VSTRIDE + hl * VSTRIDE
                        nc.tensor.matmul(
                            pso[:],
                            exps[:, kt * 1024 + sq * 128 : kt * 1024 + sq * 128 + 128],
                            v_sb[:, vb : vb + VSTRIDE],
                            start=(kt == 0),
                            stop=(kt == i),
                        )
                    recip = smallp.tile([128, 1], F32, tag="recip")
                    nc.vector.reciprocal(recip[:], pso[:, DV : DV + 1])
                    ot = otp.tile([128, DV], BF16, tag="ot")
                    nc.vector.tensor_scalar_mul(ot[:], pso[:, 0:DV], recip[:])
                    for half in range(2):
                        hc = 2 * hl + half
                        nc.sync.dma_start_transpose(
                            got_sb[:, hc * S + i * 128 : hc * S + i * 128 + 128],
                            ot[:, half * 128 : half * 128 + 128],
                        )

                for j in range(NQT2):   # 1024-wide q tiles
                    exps = expp.tile([128, 16 * 1024], BF16, tag="expS")
                    for kt in range(8 * j + 8):
                        st_tile(j, kt)
                        sq = kt - 8 * j
                        if sq >= 0:
                            av_tile(j, sq)

        # ---- gelu (exact erf) in place on transposed layout ----------
        for hc in range(NM):
            nc.scalar.activation(
                got_sb[:, hc * S : (hc + 1) * S],
                got_sb[:, hc * S : (hc + 1) * S],
                AF.Gelu,
            )

        # ---- FF partial + chunked ReduceScatter ----------------------
        with (
            tc.tile_pool(name="ffw", bufs=1) as ffwp,
            tc.tile_pool(name="ffout", bufs=4) as ffoutp,
            tc.tile_pool(name="res", bufs=2) as resp,
            tc.tile_pool(name="psFf", bufs=3, space="PSUM") as psF,
        ):
            wf_sb = load_chunked(ffwp, wf, D)
            # residual x rows: no deps, load early
            xrs = []
            for g in range(4):
                xr = resp.tile([128, D], F32, tag=f"xr{g}")
                dma.dma_start(xr[:], xres[g * 128 : (g + 1) * 128, :])
                xrs.append(xr)

            rs_ds = []
            for g in range(4):
                partial_d = dramp.tile([512, D], BF16, tag=f"part{g}")
                for cc in range(4):
                    c = 4 * g + cc
                    ps0 = psF.tile([128, 512], F32, tag="ff0")
                    ps1 = psF.tile([128, 512], F32, tag="ff1")
                    for hc in range(NM):
                        lhsT = got_sb[:, hc * S + c * 128 : hc * S + c * 128 + 128]
                        nc.tensor.matmul(
                            ps0[:], lhsT, wf_sb[:, hc * D : hc * D + 512],
                            start=(hc == 0), stop=(hc == NM - 1),
                        )
                        nc.tensor.matmul(
                            ps1[:], lhsT, wf_sb[:, hc * D + 512 : hc * D + 1024],
                            start=(hc == 0), stop=(hc == NM - 1),
                        )
                    fo = ffoutp.tile([128, D], BF16, tag="ffout")
                    nc.vector.tensor_copy(fo[:, 0:512], ps0[:])
                    nc.vector.tensor_copy(fo[:, 512:1024], ps1[:])
                    dma.dma_start(partial_d[cc * 128 : (cc + 1) * 128, :], fo[:])
                rs_d = dramp.tile([128, D], BF16, tag=f"rs{g}")
                nc.gpsimd.collective_compute(
                    "ReduceScatter",
                    mybir.AluOpType.add,
                    replica_groups=[[0, 1, 2, 3], [4, 5, 6, 7]],
                    ins=[partial_d.opt()],
                    outs=[rs_d.opt()],
                )
                rs_ds.append(rs_d)

            # ---- residual adds after all RS are queued ---------------
            for g in range(4):
                rb = resp.tile([128, D], BF16, tag=f"rb{g}")
                dma.dma_start(rb[:], rs_ds[g][:])
                rf = resp.tile([128, D], F32, tag=f"rf{g}")
                nc.vector.tensor_copy(rf[:], rb[:])
                nc.vector.tensor_add(xrs[g][:], xrs[g][:], rf[:])
                dma.dma_start(out[g * 128 : (g + 1) * 128, :], xrs[g][:])


def make_in_maps(x, Wk, bk, Wv, bv, Wf, bf):
    """Host-side sharding: returns the per-core input dict list."""
    x = np.asarray(x, np.float32)
    mask = np.tril(np.ones((128, 128), np.float32)).T  # mask[k,q]=1 iff k<=q
    in_maps = []
    for c in range(NCORES):
        b, r = c // GROUP, c % GROUP
        xb = x[b]                                    # [S, D]
        xT = np.ascontiguousarray(xb.T).astype(bf16)
        qTs = xT[DKS * r : DKS * (r + 1)]            # heads 4r..4r+3 rows
        # chunked RS: core (b,r) ends up with rows {512g+128r+[0,128)}
        xres = np.concatenate(
            [xb[512 * g + 128 * r : 512 * g + 128 * r + 128] for g in range(4)]
        ) + bf[None, :].astype(np.float32)
        in_maps.append({
            "xT": xT,
            "qT": np.ascontiguousarray(qTs),
            "xres": np.ascontiguousarray(xres),
            "wk": np.ascontiguousarray(Wk[:, DKS * r : DKS * (r + 1)]).astype(bf16),
            "wv": np.ascontiguousarray(Wv[:, DVS * r : DVS * (r + 1)]).astype(bf16),
            "wf": np.ascontiguousarray(Wf[DVS * r : DVS * (r + 1), :]).astype(bf16),
            "bkb": bk[None, DKS * r : DKS * (r + 1)].astype(bf16),
            "bvb": bv[None, DVS * r : DVS * (r + 1)].astype(bf16),
            "maskt": mask.astype(bf16),
            "ident": np.eye(128, dtype=np.float32).astype(bf16),
            "onesr": np.ones((1, 512), bf16),
        })
    return in_maps


def assemble(results):
    """[8 x [512,1024]] core outputs -> [2,2048,1024]."""
    out = np.empty((B, S, D), np.float32)
    for c in range(NCORES):
        b, r = c // GROUP, c % GROUP
        for g in range(4):
            out[b, 512 * g + 128 * r : 512 * g + 128 * r + 128, :] = results[c][
                "out"
            ][128 * g : 128 * (g + 1)]
    return out


def kernel(x, Wk, bk, Wv, bv, Wf, bf, _trace=False, _trace_cores=None):
    global _compiled
    if _compiled is None:
        _compiled = build_program()
    nc = _compiled
    in_maps = make_in_maps(x, Wk, bk, Wv, bv, Wf, bf)
    res = bass_utils.run_bass_kernel_spmd(
        nc,
        in_maps,
        core_ids=list(range(NCORES)),
        trace=_trace,
        trace_cores=_trace_cores,
    )
    out = assemble(res.results)
    kernel.last_result = res
    return out
